# revision 14
# baseline (speedup 1.0000x reference)
"""Conv2d 3x3 (B=32, Cin=128, H=W=56, Cout=256, pad=1, stride=1) + bias.

Strategy: data-parallel over batch across 8 NeuronCores (4 images/core).
Per core, 1D Winograd F(2,3) along the width axis, direct along height:
the 3 horizontal taps (6 madds per 2 outputs) become 4 transformed taps
(4 madds per 2 outputs) -> 1.5x less TensorE work than direct conv.

  d_k[y, tx] = xpad[y, 2tx+k]             (k = 0..3)
  V_0 = d0-d2  V_1 = d1+d2  V_2 = d2-d1  V_3 = d1-d3      (VectorE, fp16)
  M_j[co, y, tx] = sum_ky U_{j,ky}[ci, co]^T @ V_j[ci, y+ky, tx]  (PSUM acc)
  y_even = M_0 + M_1 + M_2 + bias
  y_odd  = M_1 - M_2 - M_3 + bias

U_{j,ky} = sum_kx G[j,kx] W[co, ci, ky, kx] is host-precomputed (fp16).

Vector-engine ops cost ~(rows x (0.5ns x elems + 25ns)): every on-device
tensor op here is a single flat contiguous run to avoid per-row overhead:
 - host pre-slices x into the four shifted planes d0..d3 (pure strided
   numpy views, zero arithmetic), each [C_IN, 58, 28] with zero-padded
   border rows/cols -> input transform is 4 flat fp16 tensor_tensor ops
   per image and the x DMA is fully contiguous;
 - V is stored flat [128, 4, 58*28] so every matmul rhs is a flat
   contiguous [128, 392] slice, and each PSUM tap is a flat [128, 392]
   slice of a bank-aligned [128, 4, 512] group tile (2 groups in flight);
 - output is written parity-planar [128, 2, 392] fp16 (even plane, odd
   plane); the host interleaves the planes and upcasts to fp32.

Drain per group: ScalarE evacuates a1 = M_1 + bias (the only engine-2
helper; ScalarE also reads PSUM), then VectorE does 4 flat tensor_tensor
ops touching one PSUM operand each (ISA limit). Output DMA rides the
otherwise-idle GpSimd queue; x on sync, weights on scalar.
"""

import numpy as np

import concourse.bass as bass
import concourse.mybir as mybir
import concourse.tile as tile
from concourse import bacc
from concourse.bass_utils import run_bass_kernel_spmd

B, C_IN, H, W = 32, 128, 56, 56
C_OUT, KSZ = 256, 3
N_CORES = 8
B_LOC = B // N_CORES  # 4 images per core
CBLKS = C_OUT // 128  # 2
RT = 14  # output rows per tile group
NT = H // RT  # 4 row tiles
NTX = W // 2  # 28 winograd tile pairs per row
TAPS = 4  # F(2,3) transformed taps
XR = H + 2  # padded rows
FV = XR * NTX  # 1624 flat V/plane elems per tap
FG = RT * NTX  # 392 flat elems per group


def build_nc():
    fp16 = mybir.dt.float16
    fp32 = mybir.dt.float32
    add = mybir.AluOpType.add
    sub = mybir.AluOpType.subtract

    nc = bacc.Bacc(None, target_bir_lowering=False)
    xs = nc.dram_tensor("xs", [B_LOC, TAPS, C_IN, FV], fp16, kind="ExternalInput")
    wt = nc.dram_tensor("wt", [C_IN, CBLKS, TAPS * KSZ, 128], fp16, kind="ExternalInput")
    bias = nc.dram_tensor("bias", [128, CBLKS], fp32, kind="ExternalInput")
    out = nc.dram_tensor("out", [B_LOC, C_OUT, 2, NT * FG], fp16, kind="ExternalOutput")

    with tile.TileContext(nc) as tc:
        with (
            tc.tile_pool(name="xin", bufs=2) as xpool,
            tc.tile_pool(name="vin", bufs=2) as vpool,
            tc.tile_pool(name="wpool", bufs=1) as wpool,
            tc.tile_pool(name="spool", bufs=4) as spool,
            tc.tile_pool(name="psum", bufs=2, space="PSUM") as psum_pool,
            tc.tile_pool(name="outp", bufs=6) as opool,
        ):
            # weights + bias on the scalar DMA ring (sync ring carries x)
            w_sb = wpool.tile([C_IN, CBLKS, TAPS * KSZ, 128], fp16)
            for cb in range(CBLKS):
                nc.scalar.dma_start(w_sb[:, cb], wt[:, cb])
            bias_sb = wpool.tile([128, CBLKS], fp32)
            nc.scalar.dma_start(bias_sb[:], bias[:, :])

            # HAM pre-warm on a memset scratch tile: absorbs the PE IRAM
            # first-fetch stall and starts the HAM ramp while the first
            # x planes are still in flight on the DMA ring.
            warm = wpool.tile([C_IN, 256], fp16)
            warm_ps = psum_pool.tile([128, TAPS, 512], fp32, name="psg")
            nc.gpsimd.memset(warm[:].bitcast(mybir.dt.uint16), 0)
            for _ in range(8):
                nc.tensor.matmul(
                    warm_ps[:, 0, 0:256], warm[:, :128], warm[:, :256],
                    start=True, stop=True, skip_group_check=True,
                )

            # split DMA + transform in row-halves so GEMMs start early
            halves = [(0, 812), (812, FV)]

            for b in range(B_LOC):
                xk = xpool.tile([C_IN, TAPS, FV], fp16)
                for f0, f1 in halves:
                    for k in range(TAPS):
                        nc.sync.dma_start(xk[:, k, f0:f1], xs[b, k, :, f0:f1])
                # input transform: V_j as flat contiguous fp16 ops,
                # split across VectorE and GpSimd (both SBUF-only here)
                v = vpool.tile([C_IN, TAPS, FV], fp16)
                for f0, f1 in halves:
                    d0 = xk[:, 0, f0:f1]
                    d1 = xk[:, 1, f0:f1]
                    d2 = xk[:, 2, f0:f1]
                    d3 = xk[:, 3, f0:f1]
                    nc.vector.tensor_tensor(v[:, 0, f0:f1], d0, d2, sub)
                    nc.gpsimd.tensor_tensor(v[:, 1, f0:f1], d1, d2, add)
                    nc.vector.tensor_tensor(v[:, 2, f0:f1], d2, d1, sub)
                    nc.gpsimd.tensor_tensor(v[:, 3, f0:f1], d1, d3, sub)

                for cb in range(CBLKS):
                    bvec = bias_sb[:, cb : cb + 1]
                    for t in range(NT):
                        # one 4-bank PSUM tile per group: tap j in bank j
                        psg = psum_pool.tile([128, TAPS, 512], fp32, name="psg")
                        ps = [psg[:, j, 0:FG] for j in range(TAPS)]
                        for j in range(TAPS):
                            for ky in range(KSZ):
                                f0 = (t * RT + ky) * NTX
                                nc.tensor.matmul(
                                    ps[j],
                                    w_sb[:, cb, j * KSZ + ky, :],
                                    v[:, j, f0 : f0 + FG],
                                    start=(ky == 0),
                                    stop=(ky == KSZ - 1),
                                    skip_group_check=True,
                                )
                        # drain. ScalarE (PSUM-capable) evacuates two taps:
                        #   a1 = M1 + bias (fp16), a2m = -M2 (fp16)
                        # GpSimd/VectorE combine them at cheap fp16 rate:
                        #   t1 = a1 - a2m = M1 + M2 + bias
                        #   t2 = a1 + a2m = M1 - M2 + bias
                        # VectorE finishes, one fp32 PSUM operand per op:
                        #   y_even = t1 + P0,  y_odd = t2 - P3
                        yt = opool.tile([128, 2, FG], fp16)
                        a1 = spool.tile([128, FG], fp16)
                        a2m = spool.tile([128, FG], fp16)
                        t1 = spool.tile([128, FG], fp32)
                        t2 = spool.tile([128, FG], fp32)
                        nc.scalar.activation(
                            a1[:], ps[1], mybir.ActivationFunctionType.Identity,
                            bias=bvec, scale=1.0,
                        )
                        nc.scalar.mul(a2m[:], ps[2], -1.0)
                        nc.gpsimd.tensor_tensor(t1[:], a1[:], a2m[:], sub)
                        nc.vector.tensor_tensor(t2[:], a1[:], a2m[:], add)
                        nc.vector.tensor_tensor(yt[:, 0, :], t1[:], ps[0], add)
                        nc.vector.tensor_tensor(yt[:, 1, :], t2[:], ps[3], sub)
                        nc.gpsimd.dma_start(
                            out[b, cb * 128 : (cb + 1) * 128, :, t * FG : (t + 1) * FG],
                            yt[:],
                        )
    nc.finalize()
    return nc


def prep_inputs(x, weight, bias):
    # U[j, ky, co, ci] = sum_kx G[j, kx] * w[co, ci, ky, kx]
    G = np.array(
        [[1, 0, 0], [0.5, 0.5, 0.5], [0.5, -0.5, 0.5], [0, 0, 1]], dtype=np.float64
    )
    U = np.einsum("jk,oiyk->jyoi", G, weight.astype(np.float64))  # [4,3,co,ci]
    wt = (
        U.reshape(TAPS * KSZ, CBLKS, 128, C_IN)
        .transpose(3, 1, 0, 2)
        .astype(np.float16)
    )
    wt = np.ascontiguousarray(wt)
    bias_r = np.ascontiguousarray(bias.reshape(CBLKS, 128).T, dtype=np.float32)

    # shifted input planes d_k[y, tx] = xpad[y, 2tx+k]: pure strided slicing
    xp = np.zeros((B, C_IN, XR, W + 2), dtype=np.float16)
    xp[:, :, 1 : 1 + H, 1 : 1 + W] = x
    xs = np.empty((B, TAPS, C_IN, XR, NTX), dtype=np.float16)
    for k in range(TAPS):
        xs[:, k] = xp[:, :, :, k : k + 2 * NTX : 2]
    xs = xs.reshape(B, TAPS, C_IN, FV)

    in_maps = []
    for c in range(N_CORES):
        in_maps.append(
            {
                "xs": np.ascontiguousarray(xs[c * B_LOC : (c + 1) * B_LOC]),
                "wt": wt,
                "bias": bias_r,
            }
        )
    return in_maps


def gather_out(res):
    # parity-planar fp16 [B_LOC, C_OUT, 2, NT*FG] -> [B, C_OUT, H, W] fp32
    o = np.concatenate([r["out"] for r in res.results], axis=0)
    o = o.reshape(B, C_OUT, 2, NT, RT, NTX)
    out = np.empty((B, C_OUT, H, W), dtype=np.float32)
    out[:, :, :, 0::2] = o[:, :, 0].reshape(B, C_OUT, H, NTX)
    out[:, :, :, 1::2] = o[:, :, 1].reshape(B, C_OUT, H, NTX)
    return out


_NC_CACHE = {}


def run(x, weight, bias, trace=False, nc=None, tmpdir=None):
    if nc is None:
        nc = _NC_CACHE.get("wino")
        if nc is None:
            nc = _NC_CACHE["wino"] = build_nc()
    in_maps = prep_inputs(np.asarray(x), np.asarray(weight), np.asarray(bias))
    res = run_bass_kernel_spmd(
        nc, in_maps, core_ids=list(range(N_CORES)), trace=trace, tmpdir=tmpdir
    )
    return gather_out(res), res


def kernel(x, weight, bias):
    out, _ = run(x, weight, bias, trace=False)
    return out


if __name__ == "__main__":
    rng = np.random.default_rng(0)
    x = rng.standard_normal((B, C_IN, H, W), dtype=np.float32)
    w = (rng.standard_normal((C_OUT, C_IN, KSZ, KSZ), dtype=np.float32) * 0.05).astype(
        np.float32
    )
    b = rng.standard_normal((C_OUT,), dtype=np.float32)
    out = kernel(x, w, b)
    print(out.shape, out.dtype)


# revision 17
# speedup vs baseline: 1.0081x; 1.0081x over previous
"""Conv2d 3x3 (B=32, Cin=128, H=W=56, Cout=256, pad=1, stride=1) + bias.

Strategy: data-parallel over batch across 8 NeuronCores (4 images/core).
Per core, 1D Winograd F(2,3) along the width axis, direct along height:
the 3 horizontal taps (6 madds per 2 outputs) become 4 transformed taps
(4 madds per 2 outputs) -> 1.5x less TensorE work than direct conv.

  d_k[y, tx] = xpad[y, 2tx+k]             (k = 0..3)
  V_0 = d0-d2  V_1 = d1+d2  V_2 = d2-d1  V_3 = d1-d3      (VectorE, fp16)
  M_j[co, y, tx] = sum_ky U_{j,ky}[ci, co]^T @ V_j[ci, y+ky, tx]  (PSUM acc)
  y_even = M_0 + M_1 + M_2 + bias
  y_odd  = M_1 - M_2 - M_3 + bias

U_{j,ky} = sum_kx G[j,kx] W[co, ci, ky, kx] is host-precomputed (fp16).

Vector-engine ops cost ~(rows x (0.5ns x elems + 25ns)): every on-device
tensor op here is a single flat contiguous run to avoid per-row overhead:
 - host pre-slices x into the four shifted planes d0..d3 (pure strided
   numpy views, zero arithmetic), each [C_IN, 58, 28] with zero-padded
   border rows/cols -> input transform is 4 flat fp16 tensor_tensor ops
   per image and the x DMA is fully contiguous;
 - V is stored flat [128, 4, 58*28] so every matmul rhs is a flat
   contiguous [128, 392] slice, and each PSUM tap is a flat [128, 392]
   slice of a bank-aligned [128, 4, 512] group tile (2 groups in flight);
 - output is written parity-planar [128, 2, 392] fp16 (even plane, odd
   plane); the host interleaves the planes and upcasts to fp32.

Drain per group: ScalarE evacuates a1 = M_1 + bias (the only engine-2
helper; ScalarE also reads PSUM), then VectorE does 4 flat tensor_tensor
ops touching one PSUM operand each (ISA limit). Output DMA rides the
otherwise-idle GpSimd queue; x on sync, weights on scalar.
"""

import numpy as np

import concourse.bass as bass
import concourse.mybir as mybir
import concourse.tile as tile
from concourse import bacc
from concourse.bass_utils import run_bass_kernel_spmd

B, C_IN, H, W = 32, 128, 56, 56
C_OUT, KSZ = 256, 3
N_CORES = 8
B_LOC = B // N_CORES  # 4 images per core
CBLKS = C_OUT // 128  # 2
RT = 14  # output rows per tile group
NT = H // RT  # 4 row tiles
NTX = W // 2  # 28 winograd tile pairs per row
TAPS = 4  # F(2,3) transformed taps
XR = H + 2  # padded rows
FV = XR * NTX  # 1624 flat V/plane elems per tap
FG = RT * NTX  # 392 flat elems per group


def build_nc():
    fp16 = mybir.dt.float16
    fp32 = mybir.dt.float32
    add = mybir.AluOpType.add
    sub = mybir.AluOpType.subtract

    nc = bacc.Bacc(None, target_bir_lowering=False)
    xs = nc.dram_tensor("xs", [B_LOC, TAPS, C_IN, FV], fp16, kind="ExternalInput")
    wt = nc.dram_tensor("wt", [C_IN, CBLKS, TAPS * KSZ, 128], fp16, kind="ExternalInput")
    bias = nc.dram_tensor("bias", [128, CBLKS], fp32, kind="ExternalInput")
    out = nc.dram_tensor("out", [B_LOC, C_OUT, 2, NT * FG], fp16, kind="ExternalOutput")

    with tile.TileContext(nc) as tc:
        with (
            tc.tile_pool(name="xin", bufs=2) as xpool,
            tc.tile_pool(name="vin", bufs=2) as vpool,
            tc.tile_pool(name="wpool", bufs=1) as wpool,
            tc.tile_pool(name="spool", bufs=4) as spool,
            tc.tile_pool(name="psum", bufs=2, space="PSUM") as psum_pool,
            tc.tile_pool(name="outp", bufs=6) as opool,
        ):
            # weights + bias on the scalar DMA ring (sync ring carries x)
            w_sb = wpool.tile([C_IN, CBLKS, TAPS * KSZ, 128], fp16)
            for cb in range(CBLKS):
                nc.scalar.dma_start(w_sb[:, cb], wt[:, cb])
            bias_sb = wpool.tile([128, CBLKS], fp32)
            nc.scalar.dma_start(bias_sb[:], bias[:, :])

            # HAM pre-warm on a memset scratch tile: absorbs the PE IRAM
            # first-fetch stall and starts the HAM ramp while the first
            # x planes are still in flight on the DMA ring.
            warm = wpool.tile([C_IN, 256], fp16)
            warm_ps = psum_pool.tile([128, TAPS, 512], fp32, name="psg")
            nc.gpsimd.memset(warm[:].bitcast(mybir.dt.uint16), 0)
            for _ in range(8):
                nc.tensor.matmul(
                    warm_ps[:, 0, 0:256], warm[:, :128], warm[:, :256],
                    start=True, stop=True, skip_group_check=True,
                )

            # split DMA + transform in row-halves so GEMMs start early
            halves = [(0, 812), (812, FV)]

            for b in range(B_LOC):
                xk = xpool.tile([C_IN, TAPS, FV], fp16)
                for f0, f1 in halves:
                    for k in range(TAPS):
                        nc.sync.dma_start(xk[:, k, f0:f1], xs[b, k, :, f0:f1])
                # input transform: V_j as flat contiguous fp16 ops,
                # split across VectorE and GpSimd (both SBUF-only here)
                v = vpool.tile([C_IN, TAPS, FV], fp16)
                for f0, f1 in halves:
                    d0 = xk[:, 0, f0:f1]
                    d1 = xk[:, 1, f0:f1]
                    d2 = xk[:, 2, f0:f1]
                    d3 = xk[:, 3, f0:f1]
                    nc.gpsimd.tensor_tensor(v[:, 0, f0:f1], d0, d2, sub)
                    nc.gpsimd.tensor_tensor(v[:, 1, f0:f1], d1, d2, add)
                    nc.gpsimd.tensor_tensor(v[:, 2, f0:f1], d2, d1, sub)
                    nc.gpsimd.tensor_tensor(v[:, 3, f0:f1], d1, d3, sub)

                for cb in range(CBLKS):
                    bvec = bias_sb[:, cb : cb + 1]
                    for t in range(NT):
                        # one 4-bank PSUM tile per group: tap j in bank j
                        psg = psum_pool.tile([128, TAPS, 512], fp32, name="psg")
                        ps = [psg[:, j, 0:FG] for j in range(TAPS)]
                        # taps 1 and 2 first so ScalarE evacuates them while
                        # taps 0 and 3 are still accumulating
                        for j in (1, 2, 0, 3):
                            for ky in range(KSZ):
                                f0 = (t * RT + ky) * NTX
                                nc.tensor.matmul(
                                    ps[j],
                                    w_sb[:, cb, j * KSZ + ky, :],
                                    v[:, j, f0 : f0 + FG],
                                    start=(ky == 0),
                                    stop=(ky == KSZ - 1),
                                    skip_group_check=True,
                                )
                        # drain. ScalarE (PSUM-capable) evacuates two taps:
                        #   a1 = M1 + bias (fp16), a2m = -M2 (fp16)
                        # GpSimd/VectorE combine them at cheap fp16 rate:
                        #   t1 = a1 - a2m = M1 + M2 + bias
                        #   t2 = a1 + a2m = M1 - M2 + bias
                        # VectorE finishes, one fp32 PSUM operand per op:
                        #   y_even = t1 + P0,  y_odd = t2 - P3
                        yt = opool.tile([128, 2, FG], fp16)
                        a1 = spool.tile([128, FG], fp16)
                        a2m = spool.tile([128, FG], fp16)
                        t1 = spool.tile([128, FG], fp16)
                        t2 = spool.tile([128, FG], fp16)
                        nc.scalar.activation(
                            a1[:], ps[1], mybir.ActivationFunctionType.Identity,
                            bias=bvec, scale=1.0,
                        )
                        nc.scalar.mul(a2m[:], ps[2], -1.0)
                        nc.vector.tensor_tensor(t1[:], a1[:], a2m[:], sub)
                        nc.vector.tensor_tensor(t2[:], a1[:], a2m[:], add)
                        nc.vector.tensor_tensor(yt[:, 0, :], t1[:], ps[0], add)
                        nc.vector.tensor_tensor(yt[:, 1, :], t2[:], ps[3], sub)
                        nc.gpsimd.dma_start(
                            out[b, cb * 128 : (cb + 1) * 128, :, t * FG : (t + 1) * FG],
                            yt[:],
                        )
    nc.finalize()
    return nc


def prep_inputs(x, weight, bias):
    # U[j, ky, co, ci] = sum_kx G[j, kx] * w[co, ci, ky, kx]
    G = np.array(
        [[1, 0, 0], [0.5, 0.5, 0.5], [0.5, -0.5, 0.5], [0, 0, 1]], dtype=np.float64
    )
    U = np.einsum("jk,oiyk->jyoi", G, weight.astype(np.float64))  # [4,3,co,ci]
    wt = (
        U.reshape(TAPS * KSZ, CBLKS, 128, C_IN)
        .transpose(3, 1, 0, 2)
        .astype(np.float16)
    )
    wt = np.ascontiguousarray(wt)
    bias_r = np.ascontiguousarray(bias.reshape(CBLKS, 128).T, dtype=np.float32)

    # shifted input planes d_k[y, tx] = xpad[y, 2tx+k]: pure strided slicing
    xp = np.zeros((B, C_IN, XR, W + 2), dtype=np.float16)
    xp[:, :, 1 : 1 + H, 1 : 1 + W] = x
    xs = np.empty((B, TAPS, C_IN, XR, NTX), dtype=np.float16)
    for k in range(TAPS):
        xs[:, k] = xp[:, :, :, k : k + 2 * NTX : 2]
    xs = xs.reshape(B, TAPS, C_IN, FV)

    in_maps = []
    for c in range(N_CORES):
        in_maps.append(
            {
                "xs": np.ascontiguousarray(xs[c * B_LOC : (c + 1) * B_LOC]),
                "wt": wt,
                "bias": bias_r,
            }
        )
    return in_maps


def gather_out(res):
    # parity-planar fp16 [B_LOC, C_OUT, 2, NT*FG] -> [B, C_OUT, H, W] fp32
    o = np.concatenate([r["out"] for r in res.results], axis=0)
    o = o.reshape(B, C_OUT, 2, NT, RT, NTX)
    out = np.empty((B, C_OUT, H, W), dtype=np.float32)
    out[:, :, :, 0::2] = o[:, :, 0].reshape(B, C_OUT, H, NTX)
    out[:, :, :, 1::2] = o[:, :, 1].reshape(B, C_OUT, H, NTX)
    return out


_NC_CACHE = {}


def run(x, weight, bias, trace=False, nc=None, tmpdir=None):
    if nc is None:
        nc = _NC_CACHE.get("wino")
        if nc is None:
            nc = _NC_CACHE["wino"] = build_nc()
    in_maps = prep_inputs(np.asarray(x), np.asarray(weight), np.asarray(bias))
    res = run_bass_kernel_spmd(
        nc, in_maps, core_ids=list(range(N_CORES)), trace=trace, tmpdir=tmpdir
    )
    return gather_out(res), res


def kernel(x, weight, bias):
    out, _ = run(x, weight, bias, trace=False)
    return out


if __name__ == "__main__":
    rng = np.random.default_rng(0)
    x = rng.standard_normal((B, C_IN, H, W), dtype=np.float32)
    w = (rng.standard_normal((C_OUT, C_IN, KSZ, KSZ), dtype=np.float32) * 0.05).astype(
        np.float32
    )
    b = rng.standard_normal((C_OUT,), dtype=np.float32)
    out = kernel(x, w, b)
    print(out.shape, out.dtype)


# revision 20
# speedup vs baseline: 1.2046x; 1.1950x over previous
"""Conv2d 3x3 (B=32, Cin=128, H=W=56, Cout=256, pad=1, stride=1) + bias.

Strategy: data-parallel over batch across 8 NeuronCores (4 images/core).
Per core, 1D Winograd F(2,3) along the width axis, direct along height:
the 3 horizontal taps (6 madds per 2 outputs) become 4 transformed taps
(4 madds per 2 outputs) -> 1.5x less TensorE work than direct conv.

  d_k[y, tx] = xpad[y, 2tx+k]             (k = 0..3)
  V_0 = d0-d2  V_1 = d1+d2  V_2 = d2-d1  V_3 = d1-d3      (VectorE, fp16)
  M_j[co, y, tx] = sum_ky U_{j,ky}[ci, co]^T @ V_j[ci, y+ky, tx]  (PSUM acc)
  y_even = M_0 + M_1 + M_2 + bias
  y_odd  = M_1 - M_2 - M_3 + bias

U_{j,ky} = sum_kx G[j,kx] W[co, ci, ky, kx] is host-precomputed (fp16).

Vector-engine ops cost ~(rows x (0.5ns x elems + 25ns)): every on-device
tensor op here is a single flat contiguous run to avoid per-row overhead:
 - host pre-slices x into the four shifted planes d0..d3 (pure strided
   numpy views, zero arithmetic), each [C_IN, 58, 28] with zero-padded
   border rows/cols -> input transform is 4 flat fp16 tensor_tensor ops
   per image and the x DMA is fully contiguous;
 - V is stored flat [128, 4, 58*28] so every matmul rhs is a flat
   contiguous [128, 392] slice, and each PSUM tap is a flat [128, 392]
   slice of a bank-aligned [128, 4, 512] group tile (2 groups in flight);
 - output is written parity-planar [128, 2, 392] fp16 (even plane, odd
   plane); the host interleaves the planes and upcasts to fp32.

Drain per group: ScalarE evacuates a1 = M_1 + bias (the only engine-2
helper; ScalarE also reads PSUM), then VectorE does 4 flat tensor_tensor
ops touching one PSUM operand each (ISA limit). Output DMA rides the
otherwise-idle GpSimd queue; x on sync, weights on scalar.
"""

import numpy as np

import concourse.bass as bass
import concourse.mybir as mybir
import concourse.tile as tile
from concourse import bacc
from concourse.bass_utils import run_bass_kernel_spmd

B, C_IN, H, W = 32, 128, 56, 56
C_OUT, KSZ = 256, 3
N_CORES = 8
B_LOC = B // N_CORES  # 4 images per core
CBLKS = C_OUT // 128  # 2
RT = 14  # output rows per tile group
NT = H // RT  # 4 row tiles
NTX = W // 2  # 28 winograd tile pairs per row
TAPS = 4  # F(2,3) transformed taps
XR = H + 2  # padded rows
FV = XR * NTX  # 1624 flat V/plane elems per tap
FG = RT * NTX  # 392 flat elems per group


def build_nc():
    fp16 = mybir.dt.float16
    fp32 = mybir.dt.float32
    add = mybir.AluOpType.add
    sub = mybir.AluOpType.subtract

    nc = bacc.Bacc(None, target_bir_lowering=False)
    xs = nc.dram_tensor("xs", [B_LOC, TAPS, C_IN, FV], fp16, kind="ExternalInput")
    wt = nc.dram_tensor("wt", [C_IN, CBLKS, TAPS * KSZ, 128], fp16, kind="ExternalInput")
    bias = nc.dram_tensor("bias", [128, CBLKS], fp32, kind="ExternalInput")
    out = nc.dram_tensor("out", [B_LOC, C_OUT, 2, NT * FG], fp16, kind="ExternalOutput")

    with tile.TileContext(nc) as tc:
        with (
            tc.tile_pool(name="xin", bufs=2) as xpool,
            tc.tile_pool(name="vin", bufs=2) as vpool,
            tc.tile_pool(name="wpool", bufs=1) as wpool,
            tc.tile_pool(name="spool", bufs=4) as spool,
            tc.tile_pool(name="psum", bufs=2, space="PSUM") as psum_pool,
            tc.tile_pool(name="outp", bufs=6) as opool,
        ):
            # weights + bias on the scalar DMA ring (sync ring carries x)
            w_sb = wpool.tile([C_IN, CBLKS, TAPS * KSZ, 128], fp16)
            for cb in range(CBLKS):
                nc.scalar.dma_start(w_sb[:, cb], wt[:, cb])
            bias_sb = wpool.tile([128, CBLKS], fp32)
            nc.scalar.dma_start(bias_sb[:], bias[:, :])

            # HAM pre-warm on a memset scratch tile: absorbs the PE IRAM
            # first-fetch stall and starts the HAM ramp while the first
            # x planes are still in flight on the DMA ring.
            warm = wpool.tile([C_IN, 256], fp16)
            warm_ps = psum_pool.tile([128, TAPS, 512], fp32, name="psg")
            nc.gpsimd.memset(warm[:].bitcast(mybir.dt.uint16), 0)
            for _ in range(12):
                nc.tensor.matmul(
                    warm_ps[:, 0, 0:256], warm[:, :128], warm[:, :256],
                    start=True, stop=True, skip_group_check=True,
                )

            # split DMA + transform in row-halves so GEMMs start early
            halves = [(0, 812), (812, FV)]

            for b in range(B_LOC):
                xk = xpool.tile([C_IN, TAPS, FV], fp16)
                for f0, f1 in halves:
                    for k in range(TAPS):
                        nc.sync.dma_start(xk[:, k, f0:f1], xs[b, k, :, f0:f1])
                # input transform: V_j as flat contiguous fp16 ops,
                # split across VectorE and GpSimd (both SBUF-only here)
                v = vpool.tile([C_IN, TAPS, FV], fp16)
                for hi, (f0, f1) in enumerate(halves):
                    d0 = xk[:, 0, f0:f1]
                    d1 = xk[:, 1, f0:f1]
                    d2 = xk[:, 2, f0:f1]
                    d3 = xk[:, 3, f0:f1]
                    # GpSimd tensor ops are ~3x slower than VectorE: give it
                    # only the two taps the first GEMMs don't gate on, and
                    # keep image 0's first half all on VectorE for startup
                    eng2 = nc.vector if (b == 0 and hi == 0) else nc.gpsimd
                    nc.vector.tensor_tensor(v[:, 1, f0:f1], d1, d2, add)
                    nc.vector.tensor_tensor(v[:, 2, f0:f1], d2, d1, sub)
                    eng2.tensor_tensor(v[:, 0, f0:f1], d0, d2, sub)
                    eng2.tensor_tensor(v[:, 3, f0:f1], d1, d3, sub)

                for cb in range(CBLKS):
                    bvec = bias_sb[:, cb : cb + 1]
                    for t in range(NT):
                        # one 4-bank PSUM tile per group: tap j in bank j
                        psg = psum_pool.tile([128, TAPS, 512], fp32, name="psg")
                        ps = [psg[:, j, 0:FG] for j in range(TAPS)]
                        # taps 1 and 2 first so ScalarE evacuates them while
                        # taps 0 and 3 are still accumulating
                        for j in (1, 2, 0, 3):
                            for ky in range(KSZ):
                                f0 = (t * RT + ky) * NTX
                                nc.tensor.matmul(
                                    ps[j],
                                    w_sb[:, cb, j * KSZ + ky, :],
                                    v[:, j, f0 : f0 + FG],
                                    start=(ky == 0),
                                    stop=(ky == KSZ - 1),
                                    skip_group_check=True,
                                )
                        # drain. ScalarE (PSUM-capable) evacuates two taps:
                        #   a1 = M1 + bias (fp16), a2m = -M2 (fp16)
                        # GpSimd/VectorE combine them at cheap fp16 rate:
                        #   t1 = a1 - a2m = M1 + M2 + bias
                        #   t2 = a1 + a2m = M1 - M2 + bias
                        # VectorE finishes, one fp32 PSUM operand per op:
                        #   y_even = t1 + P0,  y_odd = t2 - P3
                        yt = opool.tile([128, 2, FG], fp16)
                        a1 = spool.tile([128, FG], fp16)
                        a2m = spool.tile([128, FG], fp16)
                        t1 = spool.tile([128, FG], fp16)
                        t2 = spool.tile([128, FG], fp16)
                        nc.scalar.activation(
                            a1[:], ps[1], mybir.ActivationFunctionType.Identity,
                            bias=bvec, scale=1.0,
                        )
                        nc.scalar.mul(a2m[:], ps[2], -1.0)
                        nc.vector.tensor_tensor(t1[:], a1[:], a2m[:], sub)
                        nc.vector.tensor_tensor(t2[:], a1[:], a2m[:], add)
                        nc.vector.tensor_tensor(yt[:, 0, :], t1[:], ps[0], add)
                        nc.vector.tensor_tensor(yt[:, 1, :], t2[:], ps[3], sub)
                        nc.sync.dma_start(
                            out[b, cb * 128 : (cb + 1) * 128, :, t * FG : (t + 1) * FG],
                            yt[:],
                        )
    nc.finalize()
    return nc


def prep_inputs(x, weight, bias):
    # U[j, ky, co, ci] = sum_kx G[j, kx] * w[co, ci, ky, kx]
    G = np.array(
        [[1, 0, 0], [0.5, 0.5, 0.5], [0.5, -0.5, 0.5], [0, 0, 1]], dtype=np.float64
    )
    U = np.einsum("jk,oiyk->jyoi", G, weight.astype(np.float64))  # [4,3,co,ci]
    wt = (
        U.reshape(TAPS * KSZ, CBLKS, 128, C_IN)
        .transpose(3, 1, 0, 2)
        .astype(np.float16)
    )
    wt = np.ascontiguousarray(wt)
    bias_r = np.ascontiguousarray(bias.reshape(CBLKS, 128).T, dtype=np.float32)

    # shifted input planes d_k[y, tx] = xpad[y, 2tx+k]: pure strided slicing
    xp = np.zeros((B, C_IN, XR, W + 2), dtype=np.float16)
    xp[:, :, 1 : 1 + H, 1 : 1 + W] = x
    xs = np.empty((B, TAPS, C_IN, XR, NTX), dtype=np.float16)
    for k in range(TAPS):
        xs[:, k] = xp[:, :, :, k : k + 2 * NTX : 2]
    xs = xs.reshape(B, TAPS, C_IN, FV)

    in_maps = []
    for c in range(N_CORES):
        in_maps.append(
            {
                "xs": np.ascontiguousarray(xs[c * B_LOC : (c + 1) * B_LOC]),
                "wt": wt,
                "bias": bias_r,
            }
        )
    return in_maps


def gather_out(res):
    # parity-planar fp16 [B_LOC, C_OUT, 2, NT*FG] -> [B, C_OUT, H, W] fp32
    o = np.concatenate([r["out"] for r in res.results], axis=0)
    o = o.reshape(B, C_OUT, 2, NT, RT, NTX)
    out = np.empty((B, C_OUT, H, W), dtype=np.float32)
    out[:, :, :, 0::2] = o[:, :, 0].reshape(B, C_OUT, H, NTX)
    out[:, :, :, 1::2] = o[:, :, 1].reshape(B, C_OUT, H, NTX)
    return out


_NC_CACHE = {}


def run(x, weight, bias, trace=False, nc=None, tmpdir=None):
    if nc is None:
        nc = _NC_CACHE.get("wino")
        if nc is None:
            nc = _NC_CACHE["wino"] = build_nc()
    in_maps = prep_inputs(np.asarray(x), np.asarray(weight), np.asarray(bias))
    res = run_bass_kernel_spmd(
        nc, in_maps, core_ids=list(range(N_CORES)), trace=trace, tmpdir=tmpdir
    )
    return gather_out(res), res


def kernel(x, weight, bias):
    out, _ = run(x, weight, bias, trace=False)
    return out


if __name__ == "__main__":
    rng = np.random.default_rng(0)
    x = rng.standard_normal((B, C_IN, H, W), dtype=np.float32)
    w = (rng.standard_normal((C_OUT, C_IN, KSZ, KSZ), dtype=np.float32) * 0.05).astype(
        np.float32
    )
    b = rng.standard_normal((C_OUT,), dtype=np.float32)
    out = kernel(x, w, b)
    print(out.shape, out.dtype)


# revision 22
# speedup vs baseline: 1.3836x; 1.1486x over previous
"""Conv2d 3x3 (B=32, Cin=128, H=W=56, Cout=256, pad=1, stride=1) + bias.

Strategy: data-parallel over batch across 8 NeuronCores (4 images/core).
Per core, 1D Winograd F(2,3) along the width axis, direct along height:
the 3 horizontal taps (6 madds per 2 outputs) become 4 transformed taps
(4 madds per 2 outputs) -> 1.5x less TensorE work than direct conv.

  d_k[y, tx] = xpad[y, 2tx+k]             (k = 0..3)
  V_0 = d0-d2  V_1 = d1+d2  V_2 = d2-d1  V_3 = d1-d3      (VectorE, fp16)
  M_j[co, y, tx] = sum_ky U_{j,ky}[ci, co]^T @ V_j[ci, y+ky, tx]  (PSUM acc)
  y_even = M_0 + M_1 + M_2 + bias
  y_odd  = M_1 - M_2 - M_3 + bias

U_{j,ky} = sum_kx G[j,kx] W[co, ci, ky, kx] is host-precomputed (fp16).

Vector-engine ops cost ~(rows x (0.5ns x elems + 25ns)): every on-device
tensor op here is a single flat contiguous run to avoid per-row overhead:
 - host pre-slices x into the four shifted planes d0..d3 (pure strided
   numpy views, zero arithmetic), each [C_IN, 58, 28] with zero-padded
   border rows/cols -> input transform is 4 flat fp16 tensor_tensor ops
   per image and the x DMA is fully contiguous;
 - V is stored flat [128, 4, 58*28] so every matmul rhs is a flat
   contiguous [128, 392] slice, and each PSUM tap is a flat [128, 392]
   slice of a bank-aligned [128, 4, 512] group tile (2 groups in flight);
 - output is written parity-planar [128, 2, 392] fp16 (even plane, odd
   plane); the host interleaves the planes and upcasts to fp32.

Drain per group: ScalarE evacuates a1 = M_1 + bias (the only engine-2
helper; ScalarE also reads PSUM), then VectorE does 4 flat tensor_tensor
ops touching one PSUM operand each (ISA limit). Output DMA rides the
otherwise-idle GpSimd queue; x on sync, weights on scalar.
"""

import numpy as np

import concourse.bass as bass
import concourse.mybir as mybir
import concourse.tile as tile
from concourse import bacc
from concourse.bass_utils import run_bass_kernel_spmd

B, C_IN, H, W = 32, 128, 56, 56
C_OUT, KSZ = 256, 3
N_CORES = 8
B_LOC = B // N_CORES  # 4 images per core
CBLKS = C_OUT // 128  # 2
RT = 14  # output rows per tile group
NT = H // RT  # 4 row tiles
NTX = W // 2  # 28 winograd tile pairs per row
TAPS = 4  # F(2,3) transformed taps
XR = H + 2  # padded rows
FV = XR * NTX  # 1624 flat V/plane elems per tap
FG = RT * NTX  # 392 flat elems per group


def build_nc():
    fp16 = mybir.dt.float16
    fp32 = mybir.dt.float32
    add = mybir.AluOpType.add
    sub = mybir.AluOpType.subtract

    nc = bacc.Bacc(None, target_bir_lowering=False)
    xs = nc.dram_tensor("xs", [B_LOC, TAPS, C_IN, FV], fp16, kind="ExternalInput")
    wt = nc.dram_tensor("wt", [C_IN, CBLKS, TAPS * KSZ, 128], fp16, kind="ExternalInput")
    bias = nc.dram_tensor("bias", [128, CBLKS], fp32, kind="ExternalInput")
    out = nc.dram_tensor("out", [B_LOC, C_OUT, 2, NT * FG], fp16, kind="ExternalOutput")

    with tile.TileContext(nc) as tc:
        with (
            tc.tile_pool(name="xin", bufs=2) as xpool,
            tc.tile_pool(name="vin", bufs=2) as vpool,
            tc.tile_pool(name="wpool", bufs=1) as wpool,
            tc.tile_pool(name="spool", bufs=4) as spool,
            tc.tile_pool(name="psum", bufs=2, space="PSUM") as psum_pool,
            tc.tile_pool(name="outp", bufs=6) as opool,
        ):
            # weights + bias on the scalar DMA ring (sync ring carries x)
            w_sb = wpool.tile([C_IN, CBLKS, TAPS * KSZ, 128], fp16)
            for cb in range(CBLKS):
                nc.scalar.dma_start(w_sb[:, cb], wt[:, cb])
            bias_sb = wpool.tile([128, CBLKS], fp32)
            nc.scalar.dma_start(bias_sb[:], bias[:, :])

            # HAM pre-warm on a memset scratch tile: absorbs the PE IRAM
            # first-fetch stall and starts the HAM ramp while the first
            # x planes are still in flight on the DMA ring.
            warm = wpool.tile([C_IN, 256], fp16)
            warm_ps = psum_pool.tile([128, TAPS, 512], fp32, name="psg")
            nc.gpsimd.memset(warm[:].bitcast(mybir.dt.uint16), 0)
            for _ in range(12):
                nc.tensor.matmul(
                    warm_ps[:, 0, 0:256], warm[:, :128], warm[:, :256],
                    start=True, stop=True, skip_group_check=True,
                )

            # split DMA + transform in row-halves so GEMMs start early
            halves = [(0, 812), (812, FV)]

            for b in range(B_LOC):
                xk = xpool.tile([C_IN, TAPS, FV], fp16)
                for f0, f1 in halves:
                    for k in range(TAPS):
                        nc.sync.dma_start(xk[:, k, f0:f1], xs[b, k, :, f0:f1])
                # input transform: V_j as flat contiguous fp16 ops,
                # split across VectorE and GpSimd (both SBUF-only here)
                # input transform all on VectorE (GpSimd tensor ops are ~3x
                # slower, and its DMA queue must stay free of long ops)
                v = vpool.tile([C_IN, TAPS, FV], fp16)
                for f0, f1 in halves:
                    d0 = xk[:, 0, f0:f1]
                    d1 = xk[:, 1, f0:f1]
                    d2 = xk[:, 2, f0:f1]
                    d3 = xk[:, 3, f0:f1]
                    nc.vector.tensor_tensor(v[:, 1, f0:f1], d1, d2, add)
                    nc.vector.tensor_tensor(v[:, 2, f0:f1], d2, d1, sub)
                    nc.vector.tensor_tensor(v[:, 0, f0:f1], d0, d2, sub)
                    nc.vector.tensor_tensor(v[:, 3, f0:f1], d1, d3, sub)

                for cb in range(CBLKS):
                    bvec = bias_sb[:, cb : cb + 1]
                    for t in range(NT):
                        # one 4-bank PSUM tile per group: tap j in bank j
                        psg = psum_pool.tile([128, TAPS, 512], fp32, name="psg")
                        ps = [psg[:, j, 0:FG] for j in range(TAPS)]
                        # taps 1 and 2 first so ScalarE evacuates them while
                        # taps 0 and 3 are still accumulating
                        for j in (1, 2, 0, 3):
                            for ky in range(KSZ):
                                f0 = (t * RT + ky) * NTX
                                nc.tensor.matmul(
                                    ps[j],
                                    w_sb[:, cb, j * KSZ + ky, :],
                                    v[:, j, f0 : f0 + FG],
                                    start=(ky == 0),
                                    stop=(ky == KSZ - 1),
                                    skip_group_check=True,
                                )
                        # drain. ScalarE (PSUM-capable) evacuates two taps:
                        #   a1 = M1 + bias (fp16), a2m = -M2 (fp16)
                        # GpSimd/VectorE combine them at cheap fp16 rate:
                        #   t1 = a1 - a2m = M1 + M2 + bias
                        #   t2 = a1 + a2m = M1 - M2 + bias
                        # VectorE finishes, one fp32 PSUM operand per op:
                        #   y_even = t1 + P0,  y_odd = t2 - P3
                        yt = opool.tile([128, 2, FG], fp16)
                        a1 = spool.tile([128, FG], fp16)
                        a2m = spool.tile([128, FG], fp16)
                        t1 = spool.tile([128, FG], fp16)
                        t2 = spool.tile([128, FG], fp16)
                        nc.scalar.activation(
                            a1[:], ps[1], mybir.ActivationFunctionType.Identity,
                            bias=bvec, scale=1.0,
                        )
                        nc.scalar.mul(a2m[:], ps[2], -1.0)
                        nc.vector.tensor_tensor(t1[:], a1[:], a2m[:], sub)
                        nc.vector.tensor_tensor(t2[:], a1[:], a2m[:], add)
                        nc.vector.tensor_tensor(yt[:, 0, :], t1[:], ps[0], add)
                        nc.vector.tensor_tensor(yt[:, 1, :], t2[:], ps[3], sub)
                        nc.gpsimd.dma_start(
                            out[b, cb * 128 : (cb + 1) * 128, :, t * FG : (t + 1) * FG],
                            yt[:],
                        )
    nc.finalize()
    return nc


def prep_inputs(x, weight, bias):
    # U[j, ky, co, ci] = sum_kx G[j, kx] * w[co, ci, ky, kx]
    G = np.array(
        [[1, 0, 0], [0.5, 0.5, 0.5], [0.5, -0.5, 0.5], [0, 0, 1]], dtype=np.float64
    )
    U = np.einsum("jk,oiyk->jyoi", G, weight.astype(np.float64))  # [4,3,co,ci]
    wt = (
        U.reshape(TAPS * KSZ, CBLKS, 128, C_IN)
        .transpose(3, 1, 0, 2)
        .astype(np.float16)
    )
    wt = np.ascontiguousarray(wt)
    bias_r = np.ascontiguousarray(bias.reshape(CBLKS, 128).T, dtype=np.float32)

    # shifted input planes d_k[y, tx] = xpad[y, 2tx+k]: pure strided slicing
    xp = np.zeros((B, C_IN, XR, W + 2), dtype=np.float16)
    xp[:, :, 1 : 1 + H, 1 : 1 + W] = x
    xs = np.empty((B, TAPS, C_IN, XR, NTX), dtype=np.float16)
    for k in range(TAPS):
        xs[:, k] = xp[:, :, :, k : k + 2 * NTX : 2]
    xs = xs.reshape(B, TAPS, C_IN, FV)

    in_maps = []
    for c in range(N_CORES):
        in_maps.append(
            {
                "xs": np.ascontiguousarray(xs[c * B_LOC : (c + 1) * B_LOC]),
                "wt": wt,
                "bias": bias_r,
            }
        )
    return in_maps


def gather_out(res):
    # parity-planar fp16 [B_LOC, C_OUT, 2, NT*FG] -> [B, C_OUT, H, W] fp32
    o = np.concatenate([r["out"] for r in res.results], axis=0)
    o = o.reshape(B, C_OUT, 2, NT, RT, NTX)
    out = np.empty((B, C_OUT, H, W), dtype=np.float32)
    out[:, :, :, 0::2] = o[:, :, 0].reshape(B, C_OUT, H, NTX)
    out[:, :, :, 1::2] = o[:, :, 1].reshape(B, C_OUT, H, NTX)
    return out


_NC_CACHE = {}


def run(x, weight, bias, trace=False, nc=None, tmpdir=None):
    if nc is None:
        nc = _NC_CACHE.get("wino")
        if nc is None:
            nc = _NC_CACHE["wino"] = build_nc()
    in_maps = prep_inputs(np.asarray(x), np.asarray(weight), np.asarray(bias))
    res = run_bass_kernel_spmd(
        nc, in_maps, core_ids=list(range(N_CORES)), trace=trace, tmpdir=tmpdir
    )
    return gather_out(res), res


def kernel(x, weight, bias):
    out, _ = run(x, weight, bias, trace=False)
    return out


if __name__ == "__main__":
    rng = np.random.default_rng(0)
    x = rng.standard_normal((B, C_IN, H, W), dtype=np.float32)
    w = (rng.standard_normal((C_OUT, C_IN, KSZ, KSZ), dtype=np.float32) * 0.05).astype(
        np.float32
    )
    b = rng.standard_normal((C_OUT,), dtype=np.float32)
    out = kernel(x, w, b)
    print(out.shape, out.dtype)


# revision 25
# speedup vs baseline: 1.5252x; 1.1023x over previous
"""Conv2d 3x3 (B=32, Cin=128, H=W=56, Cout=256, pad=1, stride=1) + bias.

Strategy: data-parallel over batch across 8 NeuronCores (4 images/core).
Per core, 1D Winograd F(2,3) along the width axis, direct along height:
the 3 horizontal taps (6 madds per 2 outputs) become 4 transformed taps
(4 madds per 2 outputs) -> 1.5x less TensorE work than direct conv.

  d_k[y, tx] = xpad[y, 2tx+k]             (k = 0..3)
  V_0 = d0-d2  V_1 = d1+d2  V_2 = d2-d1  V_3 = d1-d3      (VectorE, fp16)
  M_j[co, y, tx] = sum_ky U_{j,ky}[ci, co]^T @ V_j[ci, y+ky, tx]  (PSUM acc)
  y_even = M_0 + M_1 + M_2 + bias
  y_odd  = M_1 - M_2 - M_3 + bias

U_{j,ky} = sum_kx G[j,kx] W[co, ci, ky, kx] is host-precomputed (fp16).

Vector-engine ops cost ~(rows x (0.5ns x elems + 25ns)): every on-device
tensor op here is a single flat contiguous run to avoid per-row overhead:
 - host pre-slices x into the four shifted planes d0..d3 (pure strided
   numpy views, zero arithmetic), each [C_IN, 58, 28] with zero-padded
   border rows/cols -> input transform is 4 flat fp16 tensor_tensor ops
   per image and the x DMA is fully contiguous;
 - V is stored flat [128, 4, 58*28] so every matmul rhs is a flat
   contiguous [128, 392] slice, and each PSUM tap is a flat [128, 392]
   slice of a bank-aligned [128, 4, 512] group tile (2 groups in flight);
 - output is written parity-planar [128, 2, 392] fp16 (even plane, odd
   plane); the host interleaves the planes and upcasts to fp32.

Drain per group: ScalarE evacuates a1 = M_1 + bias (the only engine-2
helper; ScalarE also reads PSUM), then VectorE does 4 flat tensor_tensor
ops touching one PSUM operand each (ISA limit). Output DMA rides the
otherwise-idle GpSimd queue; x on sync, weights on scalar.
"""

import numpy as np

import concourse.bass as bass
import concourse.mybir as mybir
import concourse.tile as tile
from concourse import bacc
from concourse.bass_utils import run_bass_kernel_spmd

B, C_IN, H, W = 32, 128, 56, 56
C_OUT, KSZ = 256, 3
N_CORES = 8
B_LOC = B // N_CORES  # 4 images per core
CBLKS = C_OUT // 128  # 2
RT = 14  # output rows per tile group
NT = H // RT  # 4 row tiles
NTX = W // 2  # 28 winograd tile pairs per row
TAPS = 4  # F(2,3) transformed taps
XR = H + 2  # padded rows
FV = XR * NTX  # 1624 flat V/plane elems per tap
FG = RT * NTX  # 392 flat elems per group


def build_nc():
    fp16 = mybir.dt.float16
    fp32 = mybir.dt.float32
    add = mybir.AluOpType.add
    sub = mybir.AluOpType.subtract

    nc = bacc.Bacc(None, target_bir_lowering=False)
    xs = nc.dram_tensor("xs", [B_LOC, TAPS, C_IN, FV], fp16, kind="ExternalInput")
    wt = nc.dram_tensor("wt", [C_IN, CBLKS, TAPS * KSZ, 128], fp16, kind="ExternalInput")
    bias = nc.dram_tensor("bias", [128, CBLKS], fp32, kind="ExternalInput")
    out = nc.dram_tensor("out", [B_LOC, C_OUT, 2, NT * FG], fp16, kind="ExternalOutput")

    with tile.TileContext(nc) as tc:
        with (
            tc.tile_pool(name="xin", bufs=2) as xpool,
            tc.tile_pool(name="vin", bufs=2) as vpool,
            tc.tile_pool(name="wpool", bufs=1) as wpool,
            tc.tile_pool(name="spool", bufs=4) as spool,
            tc.tile_pool(name="psum", bufs=2, space="PSUM") as psum_pool,
            tc.tile_pool(name="outp", bufs=6) as opool,
        ):
            # weights + bias on the scalar DMA ring (sync ring carries x)
            w_sb = wpool.tile([C_IN, CBLKS, TAPS * KSZ, 128], fp16)
            for cb in range(CBLKS):
                nc.scalar.dma_start(w_sb[:, cb], wt[:, cb])
            bias_sb = wpool.tile([128, CBLKS], fp32)
            nc.scalar.dma_start(bias_sb[:], bias[:, :])

            # HAM pre-warm on a memset scratch tile: absorbs the PE IRAM
            # first-fetch stall and starts the HAM ramp while the first
            # x planes are still in flight on the DMA ring.
            warm = wpool.tile([C_IN, 256], fp16)
            warm_ps = psum_pool.tile([128, TAPS, 512], fp32, name="psg")
            nc.gpsimd.memset(warm[:].bitcast(mybir.dt.uint16), 0)
            for _ in range(16):
                nc.tensor.matmul(
                    warm_ps[:, 0, 0:256], warm[:, :128], warm[:, :256],
                    start=True, stop=True, skip_group_check=True,
                )

            # split DMA + transform in row-halves so GEMMs start early
            halves = [(0, 812), (812, FV)]

            for b in range(B_LOC):
                xk = xpool.tile([C_IN, TAPS, FV], fp16)
                for f0, f1 in halves:
                    for k in (1, 2, 0, 3):  # taps 1,2 feed the first V ops
                        nc.sync.dma_start(xk[:, k, f0:f1], xs[b, k, :, f0:f1])
                # input transform: V_j as flat contiguous fp16 ops,
                # split across VectorE and GpSimd (both SBUF-only here)
                # input transform all on VectorE (GpSimd tensor ops are ~3x
                # slower, and its DMA queue must stay free of long ops)
                v = vpool.tile([C_IN, TAPS, FV], fp16)
                for f0, f1 in halves:
                    d0 = xk[:, 0, f0:f1]
                    d1 = xk[:, 1, f0:f1]
                    d2 = xk[:, 2, f0:f1]
                    d3 = xk[:, 3, f0:f1]
                    nc.vector.tensor_tensor(v[:, 1, f0:f1], d1, d2, add)
                    nc.vector.tensor_tensor(v[:, 2, f0:f1], d2, d1, sub)
                    nc.vector.tensor_tensor(v[:, 0, f0:f1], d0, d2, sub)
                    nc.vector.tensor_tensor(v[:, 3, f0:f1], d1, d3, sub)

                for cb in range(CBLKS):
                    bvec = bias_sb[:, cb : cb + 1]
                    for t in range(NT):
                        # one 4-bank PSUM tile per group: tap j in bank j
                        psg = psum_pool.tile([128, TAPS, 512], fp32, name="psg")
                        ps = [psg[:, j, 0:FG] for j in range(TAPS)]
                        # taps 1 and 2 first so ScalarE evacuates them while
                        # taps 0 and 3 are still accumulating
                        for j in (1, 2, 0, 3):
                            for ky in range(KSZ):
                                f0 = (t * RT + ky) * NTX
                                nc.tensor.matmul(
                                    ps[j],
                                    w_sb[:, cb, j * KSZ + ky, :],
                                    v[:, j, f0 : f0 + FG],
                                    start=(ky == 0),
                                    stop=(ky == KSZ - 1),
                                    skip_group_check=True,
                                )
                        # drain. ScalarE (PSUM-capable) evacuates two taps:
                        #   a1 = M1 + bias (fp16), a2m = -M2 (fp16)
                        # GpSimd/VectorE combine them at cheap fp16 rate:
                        #   t1 = a1 - a2m = M1 + M2 + bias
                        #   t2 = a1 + a2m = M1 - M2 + bias
                        # VectorE finishes, one fp32 PSUM operand per op:
                        #   y_even = t1 + P0,  y_odd = t2 - P3
                        yt = opool.tile([128, 2, FG], fp16)
                        a1 = spool.tile([128, FG], fp16)
                        a2m = spool.tile([128, FG], fp16)
                        t1 = spool.tile([128, FG], fp16)
                        t2 = spool.tile([128, FG], fp16)
                        # PSUM-touching ops first so the psg banks free as
                        # early as possible (group g+2 waits on them);
                        # SBUF-only combines finish the outputs afterwards.
                        nc.scalar.activation(
                            a1[:], ps[1], mybir.ActivationFunctionType.Identity,
                            bias=bvec, scale=1.0,
                        )
                        nc.scalar.mul(a2m[:], ps[2], -1.0)
                        nc.vector.tensor_tensor(t1[:], a1[:], ps[0], add)
                        nc.vector.tensor_tensor(t2[:], a1[:], ps[3], sub)
                        nc.vector.tensor_tensor(yt[:, 0, :], t1[:], a2m[:], sub)
                        nc.vector.tensor_tensor(yt[:, 1, :], t2[:], a2m[:], add)
                        nc.gpsimd.dma_start(
                            out[b, cb * 128 : (cb + 1) * 128, :, t * FG : (t + 1) * FG],
                            yt[:],
                        )
    nc.finalize()
    return nc


def prep_inputs(x, weight, bias):
    # U[j, ky, co, ci] = sum_kx G[j, kx] * w[co, ci, ky, kx]
    G = np.array(
        [[1, 0, 0], [0.5, 0.5, 0.5], [0.5, -0.5, 0.5], [0, 0, 1]], dtype=np.float64
    )
    U = np.einsum("jk,oiyk->jyoi", G, weight.astype(np.float64))  # [4,3,co,ci]
    wt = (
        U.reshape(TAPS * KSZ, CBLKS, 128, C_IN)
        .transpose(3, 1, 0, 2)
        .astype(np.float16)
    )
    wt = np.ascontiguousarray(wt)
    bias_r = np.ascontiguousarray(bias.reshape(CBLKS, 128).T, dtype=np.float32)

    # shifted input planes d_k[y, tx] = xpad[y, 2tx+k]: pure strided slicing
    xp = np.zeros((B, C_IN, XR, W + 2), dtype=np.float16)
    xp[:, :, 1 : 1 + H, 1 : 1 + W] = x
    xs = np.empty((B, TAPS, C_IN, XR, NTX), dtype=np.float16)
    for k in range(TAPS):
        xs[:, k] = xp[:, :, :, k : k + 2 * NTX : 2]
    xs = xs.reshape(B, TAPS, C_IN, FV)

    in_maps = []
    for c in range(N_CORES):
        in_maps.append(
            {
                "xs": np.ascontiguousarray(xs[c * B_LOC : (c + 1) * B_LOC]),
                "wt": wt,
                "bias": bias_r,
            }
        )
    return in_maps


def gather_out(res):
    # parity-planar fp16 [B_LOC, C_OUT, 2, NT*FG] -> [B, C_OUT, H, W] fp32
    o = np.concatenate([r["out"] for r in res.results], axis=0)
    o = o.reshape(B, C_OUT, 2, NT, RT, NTX)
    out = np.empty((B, C_OUT, H, W), dtype=np.float32)
    out[:, :, :, 0::2] = o[:, :, 0].reshape(B, C_OUT, H, NTX)
    out[:, :, :, 1::2] = o[:, :, 1].reshape(B, C_OUT, H, NTX)
    return out


_NC_CACHE = {}


def run(x, weight, bias, trace=False, nc=None, tmpdir=None):
    if nc is None:
        nc = _NC_CACHE.get("wino")
        if nc is None:
            nc = _NC_CACHE["wino"] = build_nc()
    in_maps = prep_inputs(np.asarray(x), np.asarray(weight), np.asarray(bias))
    res = run_bass_kernel_spmd(
        nc, in_maps, core_ids=list(range(N_CORES)), trace=trace, tmpdir=tmpdir
    )
    return gather_out(res), res


def kernel(x, weight, bias):
    out, _ = run(x, weight, bias, trace=False)
    return out


if __name__ == "__main__":
    rng = np.random.default_rng(0)
    x = rng.standard_normal((B, C_IN, H, W), dtype=np.float32)
    w = (rng.standard_normal((C_OUT, C_IN, KSZ, KSZ), dtype=np.float32) * 0.05).astype(
        np.float32
    )
    b = rng.standard_normal((C_OUT,), dtype=np.float32)
    out = kernel(x, w, b)
    print(out.shape, out.dtype)
